# revision 16
# baseline (speedup 1.0000x reference)
"""Blocked-FP8 linear (dequant + matmul + bias) on 8 Trainium2 NeuronCores.

Computation: out[b,s,o] = sum_i x[b,s,i] * (weight[o,i] * scale_inv[o//128, i//128]) + bias[o]
Shapes: x [2, 2048, 4096] f32, weight [4096, 4096] f32 (e4m3-quantized values),
        weight_scale_inv [32, 32] f32, bias [4096] f32 -> out [2, 2048, 4096] f32.

Sharding: 2-way over tokens x 4-way over out_features (colwise tensor-parallel,
no collectives). Each core computes a [2048 token, 1024 out] block as
out.T = W_deq @ X.T with K(=in_features) on the partition dim.

Host-side work is layout/sharding only: slicing, transposition to K-major,
bf16 wire format (exact for the e4m3-valued weight), and replicating the
per-block scales / bias into per-partition columns. All arithmetic (dequant,
matmul, bias) runs on device.
"""

import os
import sys

for _p in ("/opt/trn_rl_repo", "/root/.axon_site/_ro/trn_rl_repo"):
    if os.path.isdir(_p) and _p not in sys.path:
        sys.path.insert(0, _p)

import ml_dtypes
import numpy as np

import concourse.bass as bass  # noqa: F401  (registers AP machinery)
import concourse.tile as tile
from concourse import bacc, mybir
from concourse.bass_utils import run_bass_kernel_spmd
from concourse.tile import add_dep_helper

BLOCK = 128
B, S, IN, OUT = 2, 2048, 4096, 4096
N_CORES = 8
TB_SPLIT = 2            # token split
OB_SPLIT = 4            # out_features split
T_SH = B * S // TB_SPLIT    # 2048 tokens per core
O_SH = OUT // OB_SPLIT      # 1024 out features per core
KB = IN // BLOCK            # 32 k-blocks
JB = O_SH // BLOCK          # 8 local o-blocks
TT = 512                    # matmul moving free dim (tokens per psum tile)
NT = T_SH // TT             # 4 token tiles

_BF16 = ml_dtypes.bfloat16

_compiled = None


def _build_program():
    nc = bacc.Bacc("TRN2", target_bir_lowering=False, debug=False,
                   num_devices=N_CORES)

    xt = nc.dram_tensor("xt", [NT, BLOCK, KB, TT], mybir.dt.bfloat16,
                        kind="ExternalInput")
    wt = nc.dram_tensor("wt", [JB, BLOCK, KB, BLOCK], mybir.dt.bfloat16,
                        kind="ExternalInput")
    sc = nc.dram_tensor("sc", [BLOCK, JB * KB], mybir.dt.bfloat16,
                        kind="ExternalInput")
    bc = nc.dram_tensor("bc", [BLOCK, JB], mybir.dt.float32,
                        kind="ExternalInput")
    out = nc.dram_tensor("out", [O_SH, T_SH], mybir.dt.float32,
                         kind="ExternalOutput")

    out_ap = out.ap()

    with tile.TileContext(nc) as tc:
        with (
            tc.tile_pool(name="consts", bufs=1) as consts,
            tc.tile_pool(name="wpool", bufs=JB) as wpool,
            tc.tile_pool(name="xpool", bufs=3) as xpool,
            tc.tile_pool(name="opool", bufs=8) as opool,
            tc.tile_pool(name="pspool", bufs=7, space="PSUM") as pspool,
            tc.tile_pool(name="warmps", bufs=1, space="PSUM") as warmps,
        ):
            # Tiny per-partition const loads go on the SWDGE (gpsimd) queue;
            # their sub-512B descriptors would serialize an HWDGE queue for
            # ~9us ahead of the weight stream.
            sc_t = consts.tile([BLOCK, JB * KB], mybir.dt.bfloat16)
            nc.gpsimd.dma_start(out=sc_t[:], in_=sc.ap())
            bc_t = consts.tile([BLOCK, JB], mybir.dt.float32)
            nc.gpsimd.dma_start(out=bc_t[:], in_=bc.ap())

            # PE warm-up: harmless matmuls on a zeroed scratch tile while the
            # first x panel streams in, so the HAM clock-gate is already at
            # 8/8 when real matmuls start.
            warm = consts.tile([BLOCK, BLOCK], mybir.dt.bfloat16)
            nc.vector.memset(warm[:], 0.0)
            ps_warm = warmps.tile([BLOCK, BLOCK], mybir.dt.float32)
            for _ in range(96):
                nc.tensor.matmul(ps_warm[:], warm[:], warm[:],
                                 start=True, stop=True)

            # Panel 0 gates the first real matmul: split it across both HWDGE
            # rings (Scalar + Sync) to double its effective queue bandwidth.
            # Later panels stream on the Scalar ring only, leaving the Sync
            # ring to the weight stream.
            x_tiles = []
            x_dmas = []
            KH = KB // 2
            for ti in range(NT):
                x_t = xpool.tile([BLOCK, KB, TT], mybir.dt.bfloat16)
                if ti == 0:
                    nc.scalar.dma_start(out=x_t[:, 0:KH, :],
                                        in_=xt.ap()[0][:, 0:KH, :])
                    x_dmas.append(nc.sync.dma_start(out=x_t[:, KH:KB, :],
                                                    in_=xt.ap()[0][:, KH:KB, :]))
                else:
                    x_dmas.append(
                        nc.scalar.dma_start(out=x_t[:], in_=xt.ap()[ti]))
                x_tiles.append(x_t)

            # Load + dequantize the weight, one o-block slice at a time so the
            # first matmuls only wait on 1 MiB of weight traffic.
            w_tiles = []
            w_dmas = []
            for j in range(JB):
                w_t = wpool.tile([BLOCK, KB, BLOCK], mybir.dt.bfloat16)
                w_dmas.append(nc.sync.dma_start(out=w_t[:], in_=wt.ap()[j]))
                sc_b = sc_t[:, j * KB:(j + 1) * KB].unsqueeze(2).to_broadcast(
                    (BLOCK, KB, BLOCK))
                nc.vector.tensor_mul(w_t[:], w_t[:], sc_b)
                w_tiles.append(w_t)

            # Panels 1..3 are not needed until much later; hold them back so
            # the early weight slices get DMA bandwidth.
            add_dep_helper(x_dmas[1].ins, w_dmas[2].ins, sync=True,
                           reason="weight stream ahead of x prefetch")

            for ti in range(NT):
                x_t = x_tiles[ti]
                for j in range(JB):
                    ps = pspool.tile([BLOCK, TT], mybir.dt.float32)
                    for k in range(KB):
                        nc.tensor.matmul(ps[:], w_tiles[j][:, k, :],
                                         x_t[:, k, :],
                                         start=(k == 0), stop=(k == KB - 1))
                    o_t = opool.tile([BLOCK, TT], mybir.dt.float32)
                    nc.vector.tensor_scalar_add(o_t[:], ps[:],
                                                bc_t[:, j:j + 1])
                    nc.sync.dma_start(
                        out=out_ap[j * BLOCK:(j + 1) * BLOCK,
                                   ti * TT:(ti + 1) * TT],
                        in_=o_t[:])

    nc.compile()
    return nc


def _get_program():
    global _compiled
    if _compiled is None:
        _compiled = _build_program()
    return _compiled


def _shard_inputs(x, weight, weight_scale_inv, bias):
    x_flat = np.ascontiguousarray(x.reshape(B * S, IN))
    in_maps = []
    for c in range(N_CORES):
        tb, ob = divmod(c, OB_SPLIT)
        x_sh = x_flat[tb * T_SH:(tb + 1) * T_SH, :]          # [T_SH, IN]
        # xt[ti, p, k, t] = x_sh[ti*TT + t, k*128 + p] -- panel-contiguous
        xt = np.ascontiguousarray(
            x_sh.reshape(NT, TT, KB, BLOCK).transpose(0, 3, 2, 1)
        ).astype(_BF16)                                      # [NT, 128, KB, TT]

        w_sh = weight[ob * O_SH:(ob + 1) * O_SH, :]          # [O_SH, IN]
        # wt[j, p, k, o] = w_sh[j*128 + o, k*128 + p]
        wt = np.ascontiguousarray(
            w_sh.reshape(JB, BLOCK, KB, BLOCK).transpose(0, 3, 2, 1)
        ).astype(_BF16)

        s_sh = weight_scale_inv[ob * JB:(ob + 1) * JB, :]    # [JB, KB]
        sc = np.ascontiguousarray(
            np.broadcast_to(s_sh.reshape(1, JB * KB), (BLOCK, JB * KB))
        ).astype(_BF16)

        b_sh = bias[ob * O_SH:(ob + 1) * O_SH]               # [O_SH]
        bc = np.ascontiguousarray(
            b_sh.reshape(JB, BLOCK).T).astype(np.float32)    # [128, JB]

        in_maps.append({"xt": xt, "wt": wt, "sc": sc, "bc": bc})
    return in_maps


def _run(in_maps, trace=False):
    nc = _get_program()
    return run_bass_kernel_spmd(nc, in_maps, list(range(N_CORES)),
                                trace=trace)


def _assemble(results):
    out_full = np.empty((B * S, OUT), dtype=np.float32)
    for c in range(N_CORES):
        tb, ob = divmod(c, OB_SPLIT)
        out_c = np.asarray(results[c]["out"], dtype=np.float32)  # [O_SH, T_SH]
        out_full[tb * T_SH:(tb + 1) * T_SH,
                 ob * O_SH:(ob + 1) * O_SH] = out_c.T
    return out_full.reshape(B, S, OUT)


def kernel(x, weight, weight_scale_inv, bias):
    x = np.asarray(x, dtype=np.float32)
    weight = np.asarray(weight, dtype=np.float32)
    weight_scale_inv = np.asarray(weight_scale_inv, dtype=np.float32)
    bias = np.asarray(bias, dtype=np.float32)
    assert x.shape == (B, S, IN), x.shape
    assert weight.shape == (OUT, IN), weight.shape
    assert weight_scale_inv.shape == (OUT // BLOCK, IN // BLOCK)
    assert bias.shape == (OUT,)

    in_maps = _shard_inputs(x, weight, weight_scale_inv, bias)
    res = _run(in_maps)
    return _assemble(res.results)
